# revision 23
# baseline (speedup 1.0000x reference)
"""InnerAttention kernel for 8 Trainium2 NeuronCores.

Computes, per batch b:
    e = x[b] @ y[b].T          [M, N]
    p = softmax(e, axis=-1)    (over n)
    out[b] = p.T @ x[b]        [N, D]

Sharding: data-parallel over batch (B=8 -> one batch per core). Full inputs
in, full output out. The host-side wrapper pre-casts to fp16 and pre-builds
device-friendly layouts with numpy so every DMA is 128 partitions x >=2KB
contiguous per partition:
  xTs [P, 16, 8, P] fp16  (xTs[p,mi,k,c] = x[128mi+c, 128k+p]; mm1
                           stationary chunks, loaded 1 m-tile per DMA for
                           tiles 0-3 and 4 m-tiles per DMA for 4-15)
  x16 [M, D] fp16         (x natural; mm2 moving after 1/s scaling; loaded
                           4 m-tiles per DMA)
  yT  [D, N] fp16         (y transposed, mm1 moving; loaded as 4 DMAs of
                           [128, 2, 2048] = 2 d-chunks each)

HWDGE descriptor generation costs ~0.6us per dma_start and is serialized,
so the input stream is exactly 16 DMA instructions.

Per-core (M=N=2048, D=1024, P=128):
  mm1 opening: 8 PSUM accumulation chains (m-tiles 0/1 x 4 n-slices)
       interleaved k-step by k-step behind the 4 yT DMAs, so the PE
       saturates as soon as data drips in (PSUM accumulate is per-element
       has_written state, so interleaved chains to different banks are
       fine). Dummy matmuls before that burn the 1.2->2.4GHz clock ramp.
  softmax: DVE row-max over PSUM, ACT exp (bias=-max) -> p fp16 in SBUF,
       accum_out row-sum; 1/sum folded into xs = x16 * (1/s) (fp16).
  mm2: per (n-chunk 128, d-half 512) out tile: accumulate all 16
       p.T @ xs contributions in one PSUM bank; DVE and ACT copy one
       256-col half each in parallel into one staging tile, single DMA out.
"""

import numpy as np

import concourse.bacc as bacc
import concourse.mybir as mybir
import concourse.tile as tile
from concourse import bass_utils

B, M, N, D = 8, 2048, 2048, 1024
P = 128
NSLICE = 512
N_MTILES = M // P     # 16
N_DCHUNK = D // P     # 8
N_NSL = N // NSLICE   # 4
N_NCHUNK = N // P     # 16
N_DHALF = D // NSLICE  # 2
NWARM = 36
YKB = 2               # d-chunks per yT DMA
N_YB = N_DCHUNK // YKB  # 4
NSC = N_NSL - 1       # n-slices covered by the opening interleaved chains

F32 = mybir.dt.float32
FP16 = mybir.dt.float16
AX = mybir.AxisListType.X
EXP = mybir.ActivationFunctionType.Exp


def _build_nc():
    nc = bacc.Bacc("TRN2", target_bir_lowering=False, debug=False)
    xTs_d = nc.dram_tensor("xTs", [P, N_MTILES, N_DCHUNK, P], FP16,
                           kind="ExternalInput").ap()
    x16_d = nc.dram_tensor("x16", [M, D], FP16, kind="ExternalInput").ap()
    yT_d = nc.dram_tensor("yT", [D, N], FP16, kind="ExternalInput").ap()
    out_d = nc.dram_tensor("out", [N, D], F32, kind="ExternalOutput").ap()
    warm_d = nc.dram_tensor("warm_o", [P, 4], F32, kind="ExternalOutput").ap()

    with tile.TileContext(nc) as tc:
        with (
            tc.tile_pool(name="yTp", bufs=1) as yTp,
            tc.tile_pool(name="pP", bufs=1) as pPp,
            tc.tile_pool(name="xsP", bufs=1) as xsPp,
            tc.tile_pool(name="xg", bufs=3) as xgp,
            tc.tile_pool(name="work", bufs=2) as work,
            tc.tile_pool(name="stats", bufs=3) as stats,
            tc.tile_pool(name="eps", bufs=8, space="PSUM") as epsp,
        ):
            # yT big tiles: yb[j][p, kk, n] = y[n, 128*(2j+kk)+p]
            yb = [yTp.tile([P, YKB, N], FP16, tag=f"yb{j}", name=f"yb{j}")
                  for j in range(N_YB)]
            pT = [pPp.tile([P, N], FP16, tag=f"prob{mi}", name=f"prob{mi}")
                  for mi in range(N_MTILES)]
            xs = [xsPp.tile([P, D], FP16, tag=f"xsc{mi}", name=f"xsc{mi}")
                  for mi in range(N_MTILES)]

            xT0_3 = {}
            xTg = {}
            x16g = {}
            yT_src = yT_d.rearrange("(a b p) n -> a p b n", b=YKB, p=P)
            x16_src = x16_d.rearrange("(g t p) d -> g p t d", t=4, p=P)

            def get_xT(mi, k):
                if mi < 4:
                    return xT0_3[mi][:, k, :]
                return xTg[mi // 4][:, mi % 4, k, :]

            def get_x16(mi):
                return x16g[mi // 4][:, mi % 4, :]

            def emit_mm1_group(mi, ns):
                ep = epsp.tile([P, NSLICE], F32, tag="e", name="eps")
                for k in range(N_DCHUNK):
                    nc.tensor.matmul(
                        ep[:], get_xT(mi, k),
                        yb[k // YKB][:, k % YKB, ns * NSLICE:(ns + 1) * NSLICE],
                        start=(k == 0), stop=(k == N_DCHUNK - 1),
                    )
                return ep

            rinv_of = {}
            corr_of = {}
            xsA = {}

            def emit_scale(mi):
                # xs[mi] = x16[mi] * (1/sum); deferred so a late x16 DMA
                # never blocks the DVE rmax -> ACT exp chain that frees
                # PSUM banks. Opening tiles (block-max softmax) also get
                # xsA[mi] = xs[mi] * corr for the ns0..2 column blocks.
                rinv = rinv_of.pop(mi)
                if mi in corr_of:
                    sc = stats.tile([P, 1], F32, tag="scA")
                    nc.vector.tensor_mul(sc[:], rinv[:], corr_of.pop(mi)[:])
                    xsA[mi] = xsPp.tile([P, D], FP16, tag=f"xsA{mi}",
                                        name=f"xsA{mi}")
                    nc.vector.tensor_scalar_mul(xsA[mi][:], get_x16(mi), sc[:])
                nc.vector.tensor_scalar_mul(xs[mi][:], get_x16(mi), rinv[:])

            def emit_softmax(mi, eps):
                rmax4 = stats.tile([P, N_NSL], F32, tag="rmax4")
                for ns in range(N_NSL):
                    nc.vector.reduce_max(rmax4[:, ns:ns + 1], eps[ns][:], axis=AX)
                negmax = stats.tile([P, 1], F32, tag="negmax")
                rmax = stats.tile([P, 1], F32, tag="rmax")
                nc.vector.reduce_max(rmax[:], rmax4[:], axis=AX)
                nc.vector.tensor_scalar_mul(negmax[:], rmax[:], -1.0)

                s4 = stats.tile([P, N_NSL], F32, tag="s4")
                for ns in range(N_NSL):
                    c0 = ns * NSLICE
                    nc.scalar.activation(
                        pT[mi][:, c0:c0 + NSLICE], eps[ns][:], EXP,
                        bias=negmax[:], accum_out=s4[:, ns:ns + 1],
                    )
                ssum = stats.tile([P, 1], F32, tag="ssum")
                nc.vector.reduce_sum(ssum[:], s4[:], axis=AX)
                rinv = stats.tile([P, 1], F32, tag="rinv")
                nc.vector.reciprocal(rinv[:], ssum[:])
                rinv_of[mi] = rinv
                if mi - 2 >= 0:
                    emit_scale(mi - 2)

            # --- block-max softmax for the opening tiles: exp the ns0..2
            # slices as soon as their chains stop, biased by the partial
            # max m' = max(ns0..2) (values <= 1, fp16-safe). The true row
            # max m folds in later via corr = exp(m' - m): the exact sum
            # is S = s012*corr + s3, and corr rides into mm2 through
            # xsA = xs*corr used for the ns0..2 n-chunks. This frees the
            # opening PSUM banks ~2.5us earlier than a full-row max. ---
            def emit_open_head(mi, eps012):
                rmax3 = stats.tile([P, NSC], F32, tag="rmax3")
                for ns in range(NSC):
                    nc.vector.reduce_max(rmax3[:, ns:ns + 1], eps012[ns][:],
                                         axis=AX)
                mprime = stats.tile([P, 1], F32, tag="mprime")
                nc.vector.reduce_max(mprime[:], rmax3[:], axis=AX)
                negmp = stats.tile([P, 1], F32, tag="negmp")
                nc.vector.tensor_scalar_mul(negmp[:], mprime[:], -1.0)
                s4 = stats.tile([P, N_NSL], F32, tag="s4")
                for ns in range(NSC):
                    nc.scalar.activation(
                        pT[mi][:, ns * NSLICE:(ns + 1) * NSLICE],
                        eps012[ns][:], EXP,
                        bias=negmp[:], accum_out=s4[:, ns:ns + 1],
                    )
                return mprime, s4

            def emit_open_tail(mi, ep3, mprime, s4):
                r3 = stats.tile([P, 1], F32, tag="r3")
                nc.vector.reduce_max(r3[:], ep3[:], axis=AX)
                m = stats.tile([P, 1], F32, tag="mfull")
                nc.vector.tensor_max(m[:], r3[:], mprime[:])
                negm = stats.tile([P, 1], F32, tag="negm")
                nc.vector.tensor_scalar_mul(negm[:], m[:], -1.0)
                nc.scalar.activation(
                    pT[mi][:, NSC * NSLICE:], ep3[:], EXP,
                    bias=negm[:], accum_out=s4[:, NSC:NSC + 1],
                )
                corr = stats.tile([P, 1], F32, tag="corr")
                nc.scalar.activation(corr[:], mprime[:], EXP, bias=negm[:])
                s012 = stats.tile([P, 1], F32, tag="s012")
                nc.vector.reduce_sum(s012[:], s4[:, 0:NSC], axis=AX)
                sA = stats.tile([P, 1], F32, tag="sA")
                nc.vector.tensor_mul(sA[:], s012[:], corr[:])
                stot = stats.tile([P, 1], F32, tag="stot")
                nc.vector.tensor_add(stot[:], sA[:], s4[:, NSC:NSC + 1])
                rinv = stats.tile([P, 1], F32, tag="rinv")
                nc.vector.reciprocal(rinv[:], stot[:])
                rinv_of[mi] = rinv
                corr_of[mi] = corr

            # ---- clock warmup: the PE ramps 1.2->2.4GHz only after ~3us
            # of continuous work; burn the ramp on dummy matmuls during
            # the initial DMA fill. ----
            wsrc = work.tile([P, P], FP16, tag="wsrc", bufs=1)
            nc.vector.memset(wsrc[:], 0.0)
            wps = epsp.tile([P, NSLICE], F32, tag="e", name="warmps")
            for i in range(NWARM):
                nc.tensor.matmul(wps[:, 0:P], wsrc[:], wsrc[:],
                                 start=(i == 0), stop=(i == NWARM - 1))

            # ---- input DMA stream (SP HWDGE queue; data arrives strictly
            # FIFO at ~345GB/s and issue is ~0.6-2us serialized each, so
            # both the count and the order are load-bearing). ----
            def emit_xT03(mi):
                t = work.tile([P, N_DCHUNK, P], FP16, tag="xT", bufs=4)
                nc.sync.dma_start(t[:], xTs_d[:, mi, :, :])
                xT0_3[mi] = t

            def emit_xTg(g):
                xTg[g] = xgp.tile([P, 4, N_DCHUNK, P], FP16, tag="xTg",
                                  name=f"xTg{g}")
                nc.sync.dma_start(xTg[g][:], xTs_d[:, 4 * g:4 * g + 4, :, :])

            def emit_x16g(g):
                x16g[g] = xgp.tile([P, 4, D], FP16, tag="x16g", name=f"x16g{g}")
                nc.sync.dma_start(x16g[g][:], x16_src[g])

            # yb1/yb3 go out on the Activation HWDGE queue so both DGEs
            # generate descriptors concurrently during the opening fill.
            nc.sync.dma_start(yb[0][:], yT_src[0])
            nc.scalar.dma_start(yb[1][:], yT_src[1])
            nc.scalar.dma_start(yb[3][:], yT_src[3])
            emit_xT03(0)
            emit_xT03(1)
            nc.sync.dma_start(yb[2][:], yT_src[2])
            emit_xT03(2)
            emit_xT03(3)
            emit_xTg(1)
            emit_x16g(0)
            emit_xTg(2)
            emit_x16g(1)
            emit_xTg(3)
            emit_x16g(2)
            emit_x16g(3)
            # warm readback last; its sem-wait never delays real loads, and
            # the wstage copy sits after the yb DMAs on the Act queue.
            wstage = stats.tile([P, 4], F32, tag="wstage")
            nc.scalar.copy(wstage[:], wps[:, 0:4])
            nc.sync.dma_start(warm_d, wstage[:])

            # ---- opening: 6 interleaved accumulation chains (m-tiles 0/1
            # x ns 0..2) fed k-step by k-step as each yb tile arrives; the
            # ns=3 groups run as plain groups afterwards on resident data
            # while exp(0,*) frees PSUM banks for the steady state (with 8
            # chains, all stops cluster and G(2,*) stalls on bank WAW). ----
            chain = {(mi, ns): epsp.tile([P, NSLICE], F32, tag="e", name="eps")
                     for mi in (0, 1) for ns in range(NSC)}
            for j in range(N_YB):
                for mi in (0, 1):
                    for kk in range(YKB):
                        k = YKB * j + kk
                        for ns in range(NSC):
                            nc.tensor.matmul(
                                chain[(mi, ns)][:], get_xT(mi, k),
                                yb[j][:, kk, ns * NSLICE:(ns + 1) * NSLICE],
                                start=(k == 0), stop=(k == N_DCHUNK - 1),
                            )
            st0 = emit_open_head(0, [chain[(0, ns)] for ns in range(NSC)])
            ep03 = emit_mm1_group(0, N_NSL - 1)
            st1 = emit_open_head(1, [chain[(1, ns)] for ns in range(NSC)])
            ep13 = emit_mm1_group(1, N_NSL - 1)
            emit_open_tail(0, ep03, *st0)
            emit_open_tail(1, ep13, *st1)

            # ---- steady state ----
            for mi in range(2, N_MTILES):
                eps = [emit_mm1_group(mi, ns) for ns in range(N_NSL)]
                emit_softmax(mi, eps)
            emit_scale(N_MTILES - 2)
            emit_scale(N_MTILES - 1)

            # ---- mm2: accumulate all 16 p.T @ xs contributions per out
            # tile; DVE and ACT stage one 256-col half each in parallel,
            # then a single DMA writes the tile. ----
            HS = NSLICE // 2
            for nch in range(N_NCHUNK):
                for dh in range(N_DHALF):
                    ops = epsp.tile([P, NSLICE], F32, tag="e", name="ops")
                    for mi in range(N_MTILES):
                        rhs = xs[mi]
                        if mi in xsA and nch < NSC * (NSLICE // P):
                            rhs = xsA[mi]
                        nc.tensor.matmul(
                            ops[:],
                            pT[mi][:, nch * P:(nch + 1) * P],
                            rhs[:, dh * NSLICE:(dh + 1) * NSLICE],
                            start=(mi == 0), stop=(mi == N_MTILES - 1),
                        )
                    # separate half tiles: one staging tile would serialize
                    # the DVE and ACT writers (framework WAW ordering)
                    rows = slice(nch * P, (nch + 1) * P)
                    c0 = dh * NSLICE
                    hA = work.tile([P, HS], F32, tag="ostgA", bufs=4)
                    hB = work.tile([P, HS], F32, tag="ostgB", bufs=4)
                    nc.vector.tensor_copy(hA[:], ops[:, 0:HS])
                    nc.scalar.copy(hB[:], ops[:, HS:NSLICE])
                    nc.sync.dma_start(out_d[rows, c0:c0 + HS], hA[:])
                    nc.scalar.dma_start(out_d[rows, c0 + HS:c0 + NSLICE], hB[:])

    nc.compile()
    return nc


_NC_CACHE = {}


def _get_nc():
    if "nc" not in _NC_CACHE:
        _NC_CACHE["nc"] = _build_nc()
    return _NC_CACHE["nc"]


def _host_inputs(x_b: np.ndarray, y_b: np.ndarray) -> dict:
    x16 = x_b.astype(np.float16)
    xTs = np.ascontiguousarray(
        x16.reshape(N_MTILES, P, N_DCHUNK, P).transpose(3, 0, 2, 1))
    return {
        "xTs": xTs,
        "x16": np.ascontiguousarray(x16),
        "yT": np.ascontiguousarray(y_b.astype(np.float16).T),
    }


def kernel(x: np.ndarray, y: np.ndarray) -> np.ndarray:
    assert x.shape == (B, M, D) and y.shape == (B, N, D)
    nc = _get_nc()
    in_maps = [_host_inputs(x[b], y[b]) for b in range(B)]
    res = bass_utils.run_bass_kernel_spmd(nc, in_maps, core_ids=list(range(B)))
    return np.stack([res.results[b]["out"] for b in range(B)], axis=0)


# revision 25
# speedup vs baseline: 1.0222x; 1.0222x over previous
"""InnerAttention kernel for 8 Trainium2 NeuronCores.

Computes, per batch b:
    e = x[b] @ y[b].T          [M, N]
    p = softmax(e, axis=-1)    (over n)
    out[b] = p.T @ x[b]        [N, D]

Sharding: data-parallel over batch (B=8 -> one batch per core). Full inputs
in, full output out. The host-side wrapper pre-casts to fp16 and pre-builds
device-friendly layouts with numpy so every DMA is 128 partitions x >=2KB
contiguous per partition:
  xTs [P, 16, 8, P] fp16  (xTs[p,mi,k,c] = x[128mi+c, 128k+p]; mm1
                           stationary chunks, loaded 1 m-tile per DMA for
                           tiles 0-3 and 4 m-tiles per DMA for 4-15)
  x16 [M, D] fp16         (x natural; mm2 moving after 1/s scaling; loaded
                           4 m-tiles per DMA)
  yT  [D, N] fp16         (y transposed, mm1 moving; loaded as 4 DMAs of
                           [128, 2, 2048] = 2 d-chunks each)

HWDGE descriptor generation costs ~0.6us per dma_start and is serialized,
so the input stream is exactly 16 DMA instructions.

Per-core (M=N=2048, D=1024, P=128):
  mm1 opening: 8 PSUM accumulation chains (m-tiles 0/1 x 4 n-slices)
       interleaved k-step by k-step behind the 4 yT DMAs, so the PE
       saturates as soon as data drips in (PSUM accumulate is per-element
       has_written state, so interleaved chains to different banks are
       fine). Dummy matmuls before that burn the 1.2->2.4GHz clock ramp.
  softmax: DVE row-max over PSUM, ACT exp (bias=-max) -> p fp16 in SBUF,
       accum_out row-sum; 1/sum folded into xs = x16 * (1/s) (fp16).
  mm2: per (n-chunk 128, d-half 512) out tile: accumulate all 16
       p.T @ xs contributions in one PSUM bank; DVE and ACT copy one
       256-col half each in parallel into one staging tile, single DMA out.
"""

import numpy as np

import concourse.bacc as bacc
import concourse.mybir as mybir
import concourse.tile as tile
from concourse import bass_utils

B, M, N, D = 8, 2048, 2048, 1024
P = 128
NSLICE = 512
N_MTILES = M // P     # 16
N_DCHUNK = D // P     # 8
N_NSL = N // NSLICE   # 4
N_NCHUNK = N // P     # 16
N_DHALF = D // NSLICE  # 2
NWARM = 40
YKB = 2               # d-chunks per yT DMA
N_YB = N_DCHUNK // YKB  # 4
NSC = N_NSL - 1       # n-slices covered by the opening interleaved chains

F32 = mybir.dt.float32
FP16 = mybir.dt.float16
AX = mybir.AxisListType.X
EXP = mybir.ActivationFunctionType.Exp


def _build_nc():
    nc = bacc.Bacc("TRN2", target_bir_lowering=False, debug=False)
    xTs_d = nc.dram_tensor("xTs", [P, N_MTILES, N_DCHUNK, P], FP16,
                           kind="ExternalInput").ap()
    x16_d = nc.dram_tensor("x16", [M, D], FP16, kind="ExternalInput").ap()
    yT_d = nc.dram_tensor("yT", [D, N], FP16, kind="ExternalInput").ap()
    out_d = nc.dram_tensor("out", [N, D], F32, kind="ExternalOutput").ap()
    warm_d = nc.dram_tensor("warm_o", [P, 4], F32, kind="ExternalOutput").ap()

    with tile.TileContext(nc) as tc:
        with (
            tc.tile_pool(name="yTp", bufs=1) as yTp,
            tc.tile_pool(name="pP", bufs=1) as pPp,
            tc.tile_pool(name="xsP", bufs=1) as xsPp,
            tc.tile_pool(name="xg", bufs=3) as xgp,
            tc.tile_pool(name="work", bufs=2) as work,
            tc.tile_pool(name="stats", bufs=3) as stats,
            tc.tile_pool(name="eps", bufs=8, space="PSUM") as epsp,
        ):
            # yT big tiles: yb[j][p, kk, n] = y[n, 128*(2j+kk)+p]
            yb = [yTp.tile([P, YKB, N], FP16, tag=f"yb{j}", name=f"yb{j}")
                  for j in range(N_YB)]
            pT = [pPp.tile([P, N], FP16, tag=f"prob{mi}", name=f"prob{mi}")
                  for mi in range(N_MTILES)]
            xs = [xsPp.tile([P, D], FP16, tag=f"xsc{mi}", name=f"xsc{mi}")
                  for mi in range(N_MTILES)]

            xT0_3 = {}
            xTg = {}
            x16g = {}
            yT_src = yT_d.rearrange("(a b p) n -> a p b n", b=YKB, p=P)
            x16_src = x16_d.rearrange("(g t p) d -> g p t d", t=4, p=P)

            def get_xT(mi, k):
                if mi < 4:
                    return xT0_3[mi][:, k, :]
                return xTg[mi // 4][:, mi % 4, k, :]

            def get_x16(mi):
                return x16g[mi // 4][:, mi % 4, :]

            def emit_mm1_group(mi, ns):
                ep = epsp.tile([P, NSLICE], F32, tag="e", name="eps")
                for k in range(N_DCHUNK):
                    nc.tensor.matmul(
                        ep[:], get_xT(mi, k),
                        yb[k // YKB][:, k % YKB, ns * NSLICE:(ns + 1) * NSLICE],
                        start=(k == 0), stop=(k == N_DCHUNK - 1),
                    )
                return ep

            rinv_of = {}
            corr_of = {}
            xsA = {}

            def emit_scale(mi):
                # xs[mi] = x16[mi] * (1/sum); deferred so a late x16 DMA
                # never blocks the DVE rmax -> ACT exp chain that frees
                # PSUM banks. Opening tiles (block-max softmax) also get
                # xsA[mi] = xs[mi] * corr for the ns0..2 column blocks.
                rinv = rinv_of.pop(mi)
                if mi in corr_of:
                    sc = stats.tile([P, 1], F32, tag="scA")
                    nc.vector.tensor_mul(sc[:], rinv[:], corr_of.pop(mi)[:])
                    xsA[mi] = xsPp.tile([P, D], FP16, tag=f"xsA{mi}",
                                        name=f"xsA{mi}")
                    nc.vector.tensor_scalar_mul(xsA[mi][:], get_x16(mi), sc[:])
                nc.vector.tensor_scalar_mul(xs[mi][:], get_x16(mi), rinv[:])

            def emit_softmax(mi, eps):
                rmax4 = stats.tile([P, N_NSL], F32, tag="rmax4")
                for ns in range(N_NSL):
                    nc.vector.reduce_max(rmax4[:, ns:ns + 1], eps[ns][:], axis=AX)
                negmax = stats.tile([P, 1], F32, tag="negmax")
                rmax = stats.tile([P, 1], F32, tag="rmax")
                nc.vector.reduce_max(rmax[:], rmax4[:], axis=AX)
                nc.vector.tensor_scalar_mul(negmax[:], rmax[:], -1.0)

                s4 = stats.tile([P, N_NSL], F32, tag="s4")
                for ns in range(N_NSL):
                    c0 = ns * NSLICE
                    nc.scalar.activation(
                        pT[mi][:, c0:c0 + NSLICE], eps[ns][:], EXP,
                        bias=negmax[:], accum_out=s4[:, ns:ns + 1],
                    )
                ssum = stats.tile([P, 1], F32, tag="ssum")
                nc.vector.reduce_sum(ssum[:], s4[:], axis=AX)
                rinv = stats.tile([P, 1], F32, tag="rinv")
                nc.vector.reciprocal(rinv[:], ssum[:])
                rinv_of[mi] = rinv
                if mi - 2 >= 0:
                    emit_scale(mi - 2)

            # --- block-max softmax for the opening tiles: exp the ns0..2
            # slices as soon as their chains stop, biased by the partial
            # max m' = max(ns0..2) (values <= 1, fp16-safe). The true row
            # max m folds in later via corr = exp(m' - m): the exact sum
            # is S = s012*corr + s3, and corr rides into mm2 through
            # xsA = xs*corr used for the ns0..2 n-chunks. This frees the
            # opening PSUM banks ~2.5us earlier than a full-row max. ---
            def emit_open_head(mi, eps012):
                rmax3 = stats.tile([P, NSC], F32, tag="rmax3")
                for ns in range(NSC):
                    nc.vector.reduce_max(rmax3[:, ns:ns + 1], eps012[ns][:],
                                         axis=AX)
                mprime = stats.tile([P, 1], F32, tag="mprime")
                nc.vector.reduce_max(mprime[:], rmax3[:], axis=AX)
                negmp = stats.tile([P, 1], F32, tag="negmp")
                nc.vector.tensor_scalar_mul(negmp[:], mprime[:], -1.0)
                s4 = stats.tile([P, N_NSL], F32, tag="s4")
                for ns in range(NSC):
                    nc.scalar.activation(
                        pT[mi][:, ns * NSLICE:(ns + 1) * NSLICE],
                        eps012[ns][:], EXP,
                        bias=negmp[:], accum_out=s4[:, ns:ns + 1],
                    )
                return mprime, s4

            def emit_open_tail(mi, ep3, mprime, s4):
                r3 = stats.tile([P, 1], F32, tag="r3")
                nc.vector.reduce_max(r3[:], ep3[:], axis=AX)
                m = stats.tile([P, 1], F32, tag="mfull")
                nc.vector.tensor_max(m[:], r3[:], mprime[:])
                negm = stats.tile([P, 1], F32, tag="negm")
                nc.vector.tensor_scalar_mul(negm[:], m[:], -1.0)
                nc.scalar.activation(
                    pT[mi][:, NSC * NSLICE:], ep3[:], EXP,
                    bias=negm[:], accum_out=s4[:, NSC:NSC + 1],
                )
                corr = stats.tile([P, 1], F32, tag="corr")
                nc.scalar.activation(corr[:], mprime[:], EXP, bias=negm[:])
                s012 = stats.tile([P, 1], F32, tag="s012")
                nc.vector.reduce_sum(s012[:], s4[:, 0:NSC], axis=AX)
                sA = stats.tile([P, 1], F32, tag="sA")
                nc.vector.tensor_mul(sA[:], s012[:], corr[:])
                stot = stats.tile([P, 1], F32, tag="stot")
                nc.vector.tensor_add(stot[:], sA[:], s4[:, NSC:NSC + 1])
                rinv = stats.tile([P, 1], F32, tag="rinv")
                nc.vector.reciprocal(rinv[:], stot[:])
                rinv_of[mi] = rinv
                corr_of[mi] = corr

            # ---- clock warmup: the PE ramps 1.2->2.4GHz only after ~3us
            # of continuous work; burn the ramp on dummy matmuls during
            # the initial DMA fill. ----
            wsrc = work.tile([P, P], FP16, tag="wsrc", bufs=1)
            nc.vector.memset(wsrc[:], 0.0)
            wps = epsp.tile([P, NSLICE], F32, tag="e", name="warmps")
            for i in range(NWARM):
                nc.tensor.matmul(wps[:, 0:P], wsrc[:], wsrc[:],
                                 start=(i == 0), stop=(i == NWARM - 1))

            # ---- input DMA stream (SP HWDGE queue; data arrives strictly
            # FIFO at ~345GB/s and issue is ~0.6-2us serialized each, so
            # both the count and the order are load-bearing). ----
            def emit_xT03(mi):
                t = work.tile([P, N_DCHUNK, P], FP16, tag="xT", bufs=4)
                nc.sync.dma_start(t[:], xTs_d[:, mi, :, :])
                xT0_3[mi] = t

            def emit_xTg(g):
                xTg[g] = xgp.tile([P, 4, N_DCHUNK, P], FP16, tag="xTg",
                                  name=f"xTg{g}")
                nc.sync.dma_start(xTg[g][:], xTs_d[:, 4 * g:4 * g + 4, :, :])

            def emit_x16g(g):
                x16g[g] = xgp.tile([P, 4, D], FP16, tag="x16g", name=f"x16g{g}")
                nc.sync.dma_start(x16g[g][:], x16_src[g])

            # Everything on the SP queue: data arrives strictly FIFO, so a
            # single queue gives exact need-order priority (splitting the
            # opening loads across both HWDGE queues lets later loads steal
            # bandwidth from yb0/xT0 and regresses ~6us).
            nc.sync.dma_start(yb[0][:], yT_src[0])
            emit_xT03(0)
            emit_xT03(1)
            for j in range(1, N_YB):
                nc.sync.dma_start(yb[j][:], yT_src[j])
            emit_xT03(2)
            emit_xT03(3)
            emit_xTg(1)
            emit_x16g(0)
            emit_xTg(2)
            emit_x16g(1)
            emit_xTg(3)
            emit_x16g(2)
            emit_x16g(3)
            # warm readback last; its sem-wait never delays real loads, and
            # the wstage copy sits after the yb DMAs on the Act queue.
            wstage = stats.tile([P, 4], F32, tag="wstage")
            nc.scalar.copy(wstage[:], wps[:, 0:4])
            nc.sync.dma_start(warm_d, wstage[:])

            # ---- opening: 6 interleaved accumulation chains (m-tiles 0/1
            # x ns 0..2) fed k-step by k-step as each yb tile arrives; the
            # ns=3 groups run as plain groups afterwards on resident data
            # while exp(0,*) frees PSUM banks for the steady state (with 8
            # chains, all stops cluster and G(2,*) stalls on bank WAW). ----
            chain = {(mi, ns): epsp.tile([P, NSLICE], F32, tag="e", name="eps")
                     for mi in (0, 1) for ns in range(NSC)}
            for j in range(N_YB):
                for mi in (0, 1):
                    for kk in range(YKB):
                        k = YKB * j + kk
                        for ns in range(NSC):
                            nc.tensor.matmul(
                                chain[(mi, ns)][:], get_xT(mi, k),
                                yb[j][:, kk, ns * NSLICE:(ns + 1) * NSLICE],
                                start=(k == 0), stop=(k == N_DCHUNK - 1),
                            )
            st0 = emit_open_head(0, [chain[(0, ns)] for ns in range(NSC)])
            ep03 = emit_mm1_group(0, N_NSL - 1)
            st1 = emit_open_head(1, [chain[(1, ns)] for ns in range(NSC)])
            ep13 = emit_mm1_group(1, N_NSL - 1)
            emit_open_tail(0, ep03, *st0)
            emit_open_tail(1, ep13, *st1)

            # ---- steady state ----
            for mi in range(2, N_MTILES):
                eps = [emit_mm1_group(mi, ns) for ns in range(N_NSL)]
                emit_softmax(mi, eps)
            emit_scale(N_MTILES - 2)
            emit_scale(N_MTILES - 1)

            # ---- mm2: accumulate all 16 p.T @ xs contributions per out
            # tile; DVE and ACT stage one 256-col half each in parallel,
            # then a single DMA writes the tile. ----
            HS = NSLICE // 2
            for nch in range(N_NCHUNK):
                for dh in range(N_DHALF):
                    ops = epsp.tile([P, NSLICE], F32, tag="e", name="ops")
                    for mi in range(N_MTILES):
                        rhs = xs[mi]
                        if mi in xsA and nch < NSC * (NSLICE // P):
                            rhs = xsA[mi]
                        nc.tensor.matmul(
                            ops[:],
                            pT[mi][:, nch * P:(nch + 1) * P],
                            rhs[:, dh * NSLICE:(dh + 1) * NSLICE],
                            start=(mi == 0), stop=(mi == N_MTILES - 1),
                        )
                    # separate half tiles: one staging tile would serialize
                    # the DVE and ACT writers (framework WAW ordering)
                    rows = slice(nch * P, (nch + 1) * P)
                    c0 = dh * NSLICE
                    hA = work.tile([P, HS], F32, tag="ostgA", bufs=4)
                    hB = work.tile([P, HS], F32, tag="ostgB", bufs=4)
                    nc.vector.tensor_copy(hA[:], ops[:, 0:HS])
                    nc.scalar.copy(hB[:], ops[:, HS:NSLICE])
                    nc.sync.dma_start(out_d[rows, c0:c0 + HS], hA[:])
                    nc.scalar.dma_start(out_d[rows, c0 + HS:c0 + NSLICE], hB[:])

    nc.compile()
    return nc


_NC_CACHE = {}


def _get_nc():
    if "nc" not in _NC_CACHE:
        _NC_CACHE["nc"] = _build_nc()
    return _NC_CACHE["nc"]


def _host_inputs(x_b: np.ndarray, y_b: np.ndarray) -> dict:
    x16 = x_b.astype(np.float16)
    xTs = np.ascontiguousarray(
        x16.reshape(N_MTILES, P, N_DCHUNK, P).transpose(3, 0, 2, 1))
    return {
        "xTs": xTs,
        "x16": np.ascontiguousarray(x16),
        "yT": np.ascontiguousarray(y_b.astype(np.float16).T),
    }


def kernel(x: np.ndarray, y: np.ndarray) -> np.ndarray:
    assert x.shape == (B, M, D) and y.shape == (B, N, D)
    nc = _get_nc()
    in_maps = [_host_inputs(x[b], y[b]) for b in range(B)]
    res = bass_utils.run_bass_kernel_spmd(nc, in_maps, core_ids=list(range(B)))
    return np.stack([res.results[b]["out"] for b in range(B)], axis=0)
